# revision 22
# baseline (speedup 1.0000x reference)
"""Trainium2 Bass kernel for a fused multi-head-attention block.

Computes, per the reference nn.Module:
    q = Q @ WQ + bq ; k = K @ WK + bk ; v = V @ WV + bv      (per-head split)
    attn = softmax(q k^T / sqrt(dk))                          -> output #2
    ctx  = attn @ v ; out = ctx @ WO + bo + Q (residual)
    normed = LayerNorm(out) * ln_g + ln_b                     -> output #1

Sharding (8 cores, no collectives): core c handles batch b = c // 2 and
query-row half c % 2 (2048 of 4096 rows).  Each core computes q for its
rows, k/v for the whole batch, all 8 heads, its slice of attn
[8, 2048, 4096] and normed [2048, 512].  Host concatenates.

Per-core layout strategy:
  - qT [D, M] lives in SBUF with head-dim on partitions (head h occupies
    partitions (h%2)*64..+64 of tile h//2).  kT and v spill to DRAM
    scratch and stream back per head (SBUF is too small for all three).
  - scores are computed twice, once per orientation:
      [m, j] (for the DRAM attn write, rowsums via activation accum_out)
      [j, m] (for the context matmul contraction over j)
    exp() runs on ScalarE with scale=1/8 fused; softmax normalization is
    a per-partition reciprocal multiply in [m, j] and a broadcast
    multiply on the tiny ctx^T in [j, m].
"""

import numpy as np

import concourse.bass as bass
import concourse.mybir as mybir
from concourse.bacc import Bacc
from concourse.tile import TileContext

F32 = mybir.dt.float32
P = 128


def build_mha_core(M=2048, S=4096, D=512, H=8, dk=64):
    """Build the per-core Bass program.  M = query rows on this core,
    S = kv sequence length, D = d_model, H heads of width dk."""
    assert D % P == 0 and M % P == 0 and S % P == 0
    DB = D // P           # d_model partition blocks (4)
    SC = S // P           # kv chunks of 128 (32)
    HPB = P // dk         # heads per partition block (2)
    MW = min(1024, M)     # m-window for the [j, m] orientation
    JW = min(1024, S)     # j-window for the [m, j] orientation
    SW = min(2 * JW, S)   # attn staging width
    N = 512               # fp32 moving-operand max

    nc = Bacc()

    Qc = nc.dram_tensor("Qc", [M, D], F32, kind="ExternalInput")
    Kb = nc.dram_tensor("Kb", [S, D], F32, kind="ExternalInput")
    Vb = nc.dram_tensor("Vb", [S, D], F32, kind="ExternalInput")
    WQ = nc.dram_tensor("WQ", [D, D], F32, kind="ExternalInput")
    WK = nc.dram_tensor("WK", [D, D], F32, kind="ExternalInput")
    WV = nc.dram_tensor("WV", [D, D], F32, kind="ExternalInput")
    WO = nc.dram_tensor("WO", [D, D], F32, kind="ExternalInput")
    bq = nc.dram_tensor("bq", [D], F32, kind="ExternalInput")
    bk = nc.dram_tensor("bk", [D], F32, kind="ExternalInput")
    bv = nc.dram_tensor("bv", [D], F32, kind="ExternalInput")
    bo = nc.dram_tensor("bo", [D], F32, kind="ExternalInput")
    ln_g = nc.dram_tensor("ln_g", [D], F32, kind="ExternalInput")
    ln_b = nc.dram_tensor("ln_b", [D], F32, kind="ExternalInput")
    ident = nc.dram_tensor("ident", [P, P], F32, kind="ExternalInput")

    attn_o = nc.dram_tensor("attn_o", [H, M, S], F32, kind="ExternalOutput")
    normed_o = nc.dram_tensor("normed_o", [M, D], F32, kind="ExternalOutput")

    v_s = nc.dram_tensor("v_s", [S, D], F32, kind="Internal")
    kT_s = nc.dram_tensor("kT_s", [D, S], F32, kind="Internal")
    ctxT_s = nc.dram_tensor("ctxT_s", [D, M], F32, kind="Internal")

    with TileContext(nc) as tc:
        with (
            tc.tile_pool(name="persist", bufs=1) as pp,
            tc.tile_pool(name="small", bufs=3) as sm,
            tc.tile_pool(name="ps_mm", bufs=3, space="PSUM") as ps_mm,
            tc.tile_pool(name="ps_ctx", bufs=1, space="PSUM") as ps_ctx,
        ):
            # ---------------- constants ----------------
            ident_sb = pp.tile([P, P], F32, tag="ident", name="ident_sb")
            nc.sync.dma_start(out=ident_sb, in_=ident[:, :])
            ones_row = pp.tile([1, P], F32, tag="ones", name="ones_row")
            nc.vector.memset(ones_row, 1.0)
            eps_col = pp.tile([P, 1], F32, tag="eps", name="eps_col")
            nc.vector.memset(eps_col, 1e-5)

            # per-partition bias columns: bq_sb[p, i] = bq[i*128 + p]
            bq_sb = pp.tile([P, DB], F32, tag="bq", name="bq_sb")
            bk_sb = pp.tile([P, DB], F32, tag="bk", name="bk_sb")
            nc.sync.dma_start(out=bq_sb, in_=bq.rearrange("(i p) -> p i", p=P))
            nc.sync.dma_start(out=bk_sb, in_=bk.rearrange("(i p) -> p i", p=P))

            # broadcast-along-partitions constants [128, D] via ones-matmul
            def bcast_const(src, nm):
                row = sm.tile([1, D], F32, tag="brow", name=f"{nm}_row")
                nc.sync.dma_start(out=row, in_=src.rearrange("(a d) -> a d", a=1))
                ps = ps_mm.tile([P, MW], F32, tag="mm", name=f"{nm}_ps")
                for n0 in range(0, D, N):
                    w = min(N, D - n0)
                    nc.tensor.matmul(
                        ps[:, n0:n0 + w], lhsT=ones_row[0:1, 0:P],
                        rhs=row[0:1, n0:n0 + w], start=True, stop=True,
                    )
                full = pp.tile([P, D], F32, tag=nm, name=f"{nm}_bc")
                nc.vector.tensor_copy(full, ps[:, 0:D])
                return full

            bv_bc = bcast_const(bv, "bv")
            bo_bc = bcast_const(bo, "bo")
            g_bc = bcast_const(ln_g, "g")
            b_bc = bcast_const(ln_b, "b")

            # resident projected activations (kT/v spill to DRAM scratch)
            qT_t = [pp.tile([P, M], F32, tag=f"qT{i}", name=f"qT{i}") for i in range(DB)]

            # ---------------- phase 1: projections ----------------
            with (
                tc.tile_pool(name="wpool", bufs=1) as wp,
                tc.tile_pool(name="trpool", bufs=6) as tp,
            ):
                WQ_t = [wp.tile([P, D], F32, tag=f"WQ{k}", name=f"WQ{k}") for k in range(DB)]
                WK_t = [wp.tile([P, D], F32, tag=f"WK{k}", name=f"WK{k}") for k in range(DB)]
                WV_t = [wp.tile([P, D], F32, tag=f"WV{k}", name=f"WV{k}") for k in range(DB)]
                for k in range(DB):
                    nc.sync.dma_start(out=WQ_t[k], in_=WQ[k * P:(k + 1) * P, :])
                    nc.sync.dma_start(out=WK_t[k], in_=WK[k * P:(k + 1) * P, :])
                    nc.sync.dma_start(out=WV_t[k], in_=WV[k * P:(k + 1) * P, :])

                def transpose_block(src_dram, r0, rows):
                    """Load `rows` natural rows [r0:r0+rows] and return DB SBUF
                    tiles xT[k] = src[r0:r0+rows, k*128:(k+1)*128]^T  [128, rows]."""
                    nats = []
                    for ms in range(rows // P):
                        nat = tp.tile([P, D], F32, tag="nat", name="nat")
                        nc.sync.dma_start(out=nat, in_=src_dram[r0 + ms * P: r0 + (ms + 1) * P, :])
                        nats.append(nat)
                    outs = []
                    for k in range(DB):
                        tr_ps = ps_mm.tile([P, MW], F32, tag="mm", name="tr_ps")
                        for ms in range(rows // P):
                            nc.tensor.transpose(
                                tr_ps[:, ms * P:(ms + 1) * P],
                                nats[ms][:, k * P:(k + 1) * P],
                                ident_sb,
                            )
                        xt = tp.tile([P, D], F32, tag="xt", name="xt", bufs=6)
                        nc.vector.tensor_copy(xt[:, 0:rows], tr_ps[:, 0:rows])
                        outs.append(xt)
                    return outs

                # Q -> qT (SBUF resident)
                for r0 in range(0, M, D):
                    rows = min(D, M - r0)
                    QcT = transpose_block(Qc, r0, rows)
                    for i in range(DB):
                        ps_q = ps_mm.tile([P, MW], F32, tag="mm", name="ps_q")
                        for k in range(DB):
                            nc.tensor.matmul(
                                ps_q[:, 0:rows],
                                lhsT=WQ_t[k][:, i * P:(i + 1) * P],
                                rhs=QcT[k][:, 0:rows],
                                start=(k == 0), stop=(k == DB - 1),
                            )
                        nc.vector.tensor_scalar_add(
                            qT_t[i][:, r0:r0 + rows], ps_q[:, 0:rows], bq_sb[:, i:i + 1]
                        )

                # K -> kT_s (DRAM scratch, [D, S]: per-head rows contiguous)
                for r0 in range(0, S, D):
                    rows = min(D, S - r0)
                    KbT = transpose_block(Kb, r0, rows)
                    for i in range(DB):
                        ps_k = ps_mm.tile([P, MW], F32, tag="mm", name="ps_k")
                        for k in range(DB):
                            nc.tensor.matmul(
                                ps_k[:, 0:rows],
                                lhsT=WK_t[k][:, i * P:(i + 1) * P],
                                rhs=KbT[k][:, 0:rows],
                                start=(k == 0), stop=(k == DB - 1),
                            )
                        kb_b = tp.tile([P, D], F32, tag="vb", name="kb_b", bufs=3)
                        nc.vector.tensor_scalar_add(
                            kb_b[:, 0:rows], ps_k[:, 0:rows], bk_sb[:, i:i + 1]
                        )
                        nc.sync.dma_start(
                            out=kT_s[i * P:(i + 1) * P, r0:r0 + rows], in_=kb_b[:, 0:rows]
                        )

                # V -> v_s (natural layout, DRAM scratch)
                for r0 in range(0, S, D):
                    rows = min(D, S - r0)
                    VbT = transpose_block(Vb, r0, rows)
                    for ms in range(rows // P):
                        ps_v = ps_mm.tile([P, MW], F32, tag="mm", name="ps_v")
                        for k in range(DB):
                            nc.tensor.matmul(
                                ps_v[:, 0:D],
                                lhsT=VbT[k][:, ms * P:(ms + 1) * P],
                                rhs=WV_t[k],
                                start=(k == 0), stop=(k == DB - 1),
                            )
                        vb_b = tp.tile([P, D], F32, tag="vb", name="vb_b", bufs=3)
                        nc.vector.tensor_add(vb_b, ps_v[:, 0:D], bv_bc)
                        nc.sync.dma_start(
                            out=v_s[r0 + ms * P: r0 + (ms + 1) * P, :], in_=vb_b
                        )

            # ---------------- phase 2: attention, per head ----------------
            v_s_r = v_s.rearrange("(c p) e -> p c e", p=P)  # [128, SC, D]
            with (
                tc.tile_pool(name="ph2", bufs=2) as ph2,
                tc.tile_pool(name="stage", bufs=4) as st,
                tc.tile_pool(name="upool", bufs=3) as up,
            ):
                for h in range(H):
                    i, off = h // HPB, (h % HPB) * dk
                    qT_h = qT_t[i][off:off + dk, :]   # [64, M]
                    if off == 0:
                        kpair = ph2.tile([P, S], F32, tag="kth", name="kpair")
                        nc.sync.dma_start(out=kpair, in_=kT_s[i * P:(i + 1) * P, :])
                    kT_h = kpair[off:off + dk, :]     # [64, S]
                    vh = ph2.tile([P, SC * dk], F32, tag="vh", name="vh")
                    nc.sync.dma_start(
                        out=vh.rearrange("p (c d) -> p c d", d=dk),
                        in_=v_s_r[:, :, h * dk:(h + 1) * dk],
                    )
                    recipT = ph2.tile([1, M], F32, tag="recipT", name="recipT")

                    # ---- orientation [m, j]: attn output + rowsums ----
                    for mt in range(M // P):
                        acc = sm.tile([P, S // JW], F32, tag="acc", name="acc")
                        stages = []
                        for jw in range(S // JW):
                            ps_b = ps_mm.tile([P, JW], F32, tag="mm", name="ps_b")
                            for n0 in range(0, JW, N):
                                w = min(N, JW - n0)
                                nc.tensor.matmul(
                                    ps_b[:, n0:n0 + w],
                                    lhsT=qT_h[:, mt * P:(mt + 1) * P],
                                    rhs=kT_h[:, jw * JW + n0: jw * JW + n0 + w],
                                    start=True, stop=True,
                                )
                            j0 = jw * JW
                            if j0 % SW == 0:
                                stg = st.tile([P, SW], F32, tag="stage", name="stg")
                                stages.append(stg)
                            nc.scalar.activation(
                                stages[-1][:, j0 % SW: j0 % SW + JW],
                                ps_b,
                                mybir.ActivationFunctionType.Exp,
                                scale=0.125,
                                accum_out=acc[:, jw:jw + 1],
                            )
                        rowsum = sm.tile([P, 1], F32, tag="rowsum", name="rowsum")
                        nc.vector.tensor_reduce(rowsum, acc, axis=mybir.AxisListType.X, op=mybir.AluOpType.add)
                        recip = sm.tile([P, 1], F32, tag="recip", name="recip")
                        nc.vector.reciprocal(recip, rowsum)
                        # transposed reciprocal row for the ctx normalization
                        psr = ps_mm.tile([1, P], F32, tag="mm", name="psr")
                        nc.tensor.transpose(psr, recip, ident_sb)
                        nc.vector.tensor_copy(recipT[0:1, mt * P:(mt + 1) * P], psr)
                        for si, stg in enumerate(stages):
                            nc.vector.tensor_scalar_mul(stg, stg, recip)
                            nc.sync.dma_start(
                                out=attn_o[h, mt * P:(mt + 1) * P, si * SW:(si + 1) * SW],
                                in_=stg,
                            )

                    # ---- orientation [j, m]: context ----
                    for mt in range(M // MW):
                        ps_c = ps_ctx.tile([dk, MW], F32, tag="ctx", name="ps_c")
                        for c in range(SC):
                            ps_s = ps_mm.tile([P, MW], F32, tag="mm", name="ps_s")
                            for n0 in range(0, MW, N):
                                w = min(N, MW - n0)
                                nc.tensor.matmul(
                                    ps_s[:, n0:n0 + w],
                                    lhsT=kT_h[:, c * P:(c + 1) * P],
                                    rhs=qT_h[:, mt * MW + n0: mt * MW + n0 + w],
                                    start=True, stop=True,
                                )
                            u = up.tile([P, MW], F32, tag="u", name="u")
                            nc.scalar.activation(u, ps_s, mybir.ActivationFunctionType.Exp, scale=0.125)
                            for n0 in range(0, MW, N):
                                w = min(N, MW - n0)
                                nc.tensor.matmul(
                                    ps_c[:, n0:n0 + w],
                                    lhsT=vh[:, c * dk:(c + 1) * dk],
                                    rhs=u[:, n0:n0 + w],
                                    start=(c == 0), stop=(c == SC - 1),
                                )
                        ps_rb = ps_mm.tile([P, MW], F32, tag="mm", name="ps_rb")
                        for n0 in range(0, MW, N):
                            w = min(N, MW - n0)
                            nc.tensor.matmul(
                                ps_rb[0:dk, n0:n0 + w], lhsT=ones_row[0:1, 0:dk],
                                rhs=recipT[0:1, mt * MW + n0: mt * MW + n0 + w],
                                start=True, stop=True,
                            )
                        rb = up.tile([dk, MW], F32, tag="rb", name="rb", bufs=2)
                        nc.vector.tensor_copy(rb, ps_rb[0:dk, 0:MW])
                        ctxb = up.tile([dk, MW], F32, tag="ctxb", name="ctxb", bufs=2)
                        nc.vector.tensor_mul(ctxb, ps_c, rb)
                        nc.sync.dma_start(
                            out=ctxT_s[h * dk:(h + 1) * dk, mt * MW:(mt + 1) * MW], in_=ctxb
                        )

            # ---------------- phase 3: output projection + layernorm ----------------
            with (
                tc.tile_pool(name="p3", bufs=1) as p3,
                tc.tile_pool(name="p3w", bufs=2) as wk,
            ):
                WO_t = [p3.tile([P, D], F32, tag=f"WO{k}", name=f"WO{k}") for k in range(DB)]
                ct_t = [p3.tile([P, M], F32, tag=f"ct{k}", name=f"ct{k}") for k in range(DB)]
                for k in range(DB):
                    nc.sync.dma_start(out=WO_t[k], in_=WO[k * P:(k + 1) * P, :])
                    nc.sync.dma_start(out=ct_t[k], in_=ctxT_s[k * P:(k + 1) * P, :])
                for mt in range(M // P):
                    ps_o = ps_mm.tile([P, MW], F32, tag="mm", name="ps_o")
                    for k in range(DB):
                        nc.tensor.matmul(
                            ps_o[:, 0:D],
                            lhsT=ct_t[k][:, mt * P:(mt + 1) * P],
                            rhs=WO_t[k],
                            start=(k == 0), stop=(k == DB - 1),
                        )
                    qres = wk.tile([P, D], F32, tag="qres", name="qres")
                    nc.sync.dma_start(out=qres, in_=Qc[mt * P:(mt + 1) * P, :])
                    x1 = wk.tile([P, D], F32, tag="x1", name="x1")
                    nc.vector.tensor_add(x1, ps_o[:, 0:D], qres)
                    x2 = wk.tile([P, D], F32, tag="x2", name="x2")
                    nc.vector.tensor_add(x2, x1, bo_bc)
                    s1 = sm.tile([P, 1], F32, tag="s1", name="s1")
                    nc.vector.tensor_reduce(s1, x2, axis=mybir.AxisListType.X, op=mybir.AluOpType.add)
                    nmean = sm.tile([P, 1], F32, tag="nmean", name="nmean")
                    nc.vector.tensor_scalar_mul(nmean, s1, -1.0 / D)
                    xc = wk.tile([P, D], F32, tag="xc", name="xc")
                    nc.vector.tensor_scalar_add(xc, x2, nmean)
                    sq = wk.tile([P, D], F32, tag="sq", name="sq")
                    nc.vector.tensor_mul(sq, xc, xc)
                    vsum = sm.tile([P, 1], F32, tag="vsum", name="vsum")
                    nc.vector.tensor_reduce(vsum, sq, axis=mybir.AxisListType.X, op=mybir.AluOpType.add)
                    std = sm.tile([P, 1], F32, tag="std", name="std")
                    nc.scalar.activation(
                        std, vsum, mybir.ActivationFunctionType.Sqrt,
                        bias=eps_col[:, 0:1], scale=1.0 / D,
                    )
                    rstd = sm.tile([P, 1], F32, tag="rstd", name="rstd")
                    nc.vector.reciprocal(rstd, std)
                    xn = wk.tile([P, D], F32, tag="xn", name="xn")
                    nc.vector.tensor_scalar_mul(xn, xc, rstd)
                    y1 = wk.tile([P, D], F32, tag="y1", name="y1")
                    nc.vector.tensor_mul(y1, xn, g_bc)
                    y2 = wk.tile([P, D], F32, tag="y2", name="y2")
                    nc.vector.tensor_add(y2, y1, b_bc)
                    nc.sync.dma_start(out=normed_o[mt * P:(mt + 1) * P, :], in_=y2)

    if not nc.is_finalized():
        nc.finalize()
    return nc


_NC_CACHE = {}
LAST_RESULT = None  # BassKernelResults of the most recent kernel() call


def _get_core(M, S, D, H, dk):
    key = (M, S, D, H, dk)
    if key not in _NC_CACHE:
        _NC_CACHE[key] = build_mha_core(M, S, D, H, dk)
    return _NC_CACHE[key]


def kernel(Q, K, V, WQ_w, WQ_b, WK_w, WK_b, WV_w, WV_b, WO_w, WO_b, ln_g, ln_b):
    from concourse import bass_utils

    B, S, D = Q.shape
    H, dk = 8, 64
    n_cores = 8
    halves = n_cores // B          # 2 query-halves per batch
    M = S // halves                # 2048 rows per core

    nc = _get_core(M, S, D, H, dk)

    f = np.float32
    common = {
        "WQ": np.ascontiguousarray(WQ_w, f), "bq": np.ascontiguousarray(WQ_b, f),
        "WK": np.ascontiguousarray(WK_w, f), "bk": np.ascontiguousarray(WK_b, f),
        "WV": np.ascontiguousarray(WV_w, f), "bv": np.ascontiguousarray(WV_b, f),
        "WO": np.ascontiguousarray(WO_w, f), "bo": np.ascontiguousarray(WO_b, f),
        "ln_g": np.ascontiguousarray(ln_g, f), "ln_b": np.ascontiguousarray(ln_b, f),
        "ident": np.eye(P, dtype=f),
    }
    in_maps = []
    for c in range(n_cores):
        b, half = c // halves, c % halves
        in_maps.append({
            "Qc": np.ascontiguousarray(Q[b, half * M:(half + 1) * M, :], f),
            "Kb": np.ascontiguousarray(K[b], f),
            "Vb": np.ascontiguousarray(V[b], f),
            **common,
        })

    res = bass_utils.run_bass_kernel_spmd(nc, in_maps, core_ids=list(range(n_cores)))
    global LAST_RESULT
    LAST_RESULT = res

    normed = np.empty((B, S, D), f)
    attn = np.empty((B, H, S, S), f)
    for c in range(n_cores):
        b, half = c // halves, c % halves
        out = res.results[c]
        normed[b, half * M:(half + 1) * M, :] = out["normed_o"]
        attn[b, :, half * M:(half + 1) * M, :] = out["attn_o"]
    return normed, attn
